# revision 7
# baseline (speedup 1.0000x reference)
"""Entmax-alpha Bass kernel v2: fp16-resident Newton-on-g, per-head eval counts.

Algorithm per row (K=1024, s=alpha-1, p=1/s, p*s==1):
  Solve S(t) = sum_k (s*(x_k-t))_+^p = 1 by Newton iteration on
  g(t) = S^(1/p) = s*||(x-t)_+||_p, which is CONVEX in t (norm of affine),
  so Newton from the left never overshoots:
     t+ = t + (S - S^(1-1/p)) / (s*D),   D = sum_k (s*u_k)^(p-1)
  Evals run on SBUF-resident fp16 x (read HBM once, write y once).
  Eval: u=max(x16-t,0) [DVE 4x] -> L=ln(s*u) [ACT] -> v=exp((p-1)L) [ACT]
        -> vu=v*u [Pool] -> S,D row sums [DVE fp16 fold tree].
  Dead elements: ln(0)=-inf -> v=w=0 exactly (alpha<2 strictly => p-1>0).
  Final eval computes w=exp(p*L), y = w/S streamed out via f32 staging.

Per-head eval count E in [2..7] calibrated at runtime on a host sample of the
actual inputs (device arithmetic emulated in numpy); head->core assignment is
rebalanced so all cores carry the identical E multiset (classes padded to an
even head count), letting one SPMD program serve all 8 cores.
"""

import numpy as np

import concourse.bacc as bacc
import concourse.mybir as mybir
from concourse.tile import TileContext
from concourse.bass_utils import run_bass_kernel_spmd

B, H, Q, K = 4, 16, 1024, 1024
NCORES = 8
BLOCKS = (B * H) // NCORES      # blocks per core (8)
R = 4                           # q-subrows per partition per supertile
ST_ROWS = 128 * R               # 512 rows per supertile
N_ST = BLOCKS * Q // ST_ROWS    # 16 supertiles per core
NC = N_ST * R                   # 64 state columns per core
MARGIN = 4e-3                   # fp16-rounding margin on bracket endpoints

AL = mybir.AluOpType
AF = mybir.ActivationFunctionType
F32 = mybir.dt.float32
F16 = mybir.dt.float16
U8 = mybir.dt.uint8

LAST_RESULT = None
_NC_CACHE = {}

F = np.float32
H16 = np.float16


# ---------------------------------------------------------------- calibration

def _ref_rows(X, s, p, iters=60):
    mx = X.max(-1)
    lo = mx - 1.0 / s
    hi = mx - ((1.0 / K) ** s) / s
    for _ in range(iters):
        mid = 0.5 * (lo + hi)
        Sv = ((s * np.maximum(X - mid[:, None], 0.0)) ** p).sum(-1)
        lo = np.where(Sv >= 1, mid, lo)
        hi = np.where(Sv >= 1, hi, mid)
    t = 0.5 * (lo + hi)
    w = (s * np.maximum(X - t[:, None], 0.0)) ** p
    return w / w.sum(-1, keepdims=True)


def _emul_newton(X, s_v, p_v, n_evals):
    """Numpy emulation of the device iteration (fp16 x/u/v/w, f32 sums)."""
    x16 = X.astype(H16)
    s = F(s_v)
    p = F(p_v)
    mx = x16.astype(F).max(-1, keepdims=True).astype(F)
    lo = (mx - (1.0 / s + MARGIN)).astype(F)
    hi = (mx - (((1.0 / K) ** s) / s - MARGIN)).astype(F)
    t = lo.copy()
    for e in range(n_evals):
        last = e == n_evals - 1
        u = np.maximum(x16.astype(F) - t, 0.0).astype(H16)
        with np.errstate(divide='ignore', invalid='ignore'):
            L = np.log((s * u.astype(F)).astype(F)).astype(F)
        if last:
            w16 = np.exp((p * L).astype(F)).astype(H16)
            S = w16.astype(F).sum(-1, keepdims=True).astype(F)
            return (w16.astype(F) / S).astype(F)
        v16 = np.exp(((p - 1.0) * L).astype(F)).astype(H16)
        D = v16.astype(F).sum(-1, keepdims=True).astype(F)
        vu = (v16.astype(F) * u.astype(F)).astype(H16)
        S = (s * vu.astype(F).sum(-1, keepdims=True)).astype(F)
        h = np.log(np.maximum(S, F(1e-38))).astype(F)
        if e == 0:
            hi = np.minimum(hi, (t + h).astype(F)).astype(F)
        pos = h >= 0
        lo = np.where(pos, t, lo).astype(F)
        hi = np.where(pos, hi, t).astype(F)
        Spow = np.exp(((1.0 - 1.0 / p) * h).astype(F)).astype(F)
        step = ((S - Spow) / np.maximum(s * D, F(1e-30))).astype(F)
        t = np.clip((t + step).astype(F), lo, np.maximum(hi, lo)).astype(F)
    raise AssertionError


def _choose_E(att, alpha):
    """Per-head eval count from a host sample, classes padded to even size."""
    NS = 64
    refs, errs = [], {}
    Xs = [np.ascontiguousarray(att[0, h, :NS]).astype(F) for h in range(H)]
    emax = 0.0
    for h in range(H):
        s = float(alpha[h] - 1.0)
        refs.append(_ref_rows(Xs[h].astype(np.float64), s, 1.0 / s))
        emax = max(emax, np.abs(refs[h]).max())
    thresh = 2.5e-3 * emax
    E = np.zeros(H, np.int64)
    for h in range(H):
        s = float(alpha[h] - 1.0)
        for e in range(2, 8):
            y = _emul_newton(Xs[h], s, 1.0 / s, e)
            err = np.abs(y - refs[h]).max()
            if err <= thresh or e == 7:
                E[h] = e
                errs[h] = err
                break
    # pad classes to even head counts (so 4*n_heads blocks divide over 8 cores)
    for _ in range(16):
        vals = sorted(set(E.tolist()))
        odd = [e for e in vals if (E == e).sum() % 2 == 1]
        if not odd:
            break
        e = odd[0]
        cand = [h for h in range(H) if E[h] == e]
        h = max(cand, key=lambda hh: errs.get(hh, 0.0))
        E[h] = e + 1
    return E


# ------------------------------------------------------------------- builder

def _build(e_slots):
    """e_slots: tuple of 16 per-supertile eval counts (desc, block pairs)."""
    nc = bacc.Bacc(None, target_bir_lowering=False)
    x_in = nc.declare_dram_parameter("x", [BLOCKS * Q, K], F32, isOutput=False)
    cst_in = nc.declare_dram_parameter("cst", [128, 8 * NC], F32, isOutput=False)
    y_out = nc.declare_dram_parameter("y", [BLOCKS * Q, K], F32, isOutput=True)

    emax_r = max(e_slots)
    groups = [list(range(0, N_ST // 2)), list(range(N_ST // 2, N_ST))]

    with TileContext(nc) as tc:
        with tc.tile_pool(name="state", bufs=1) as stp, \
             tc.tile_pool(name="xr", bufs=1) as xrp, \
             tc.tile_pool(name="stg", bufs=3) as sgp, \
             tc.tile_pool(name="u16", bufs=2) as up, \
             tc.tile_pool(name="lnp", bufs=1) as lp, \
             tc.tile_pool(name="vp", bufs=2) as vp_, \
             tc.tile_pool(name="vup", bufs=1) as vup, \
             tc.tile_pool(name="l1", bufs=1) as l1p, \
             tc.tile_pool(name="l2", bufs=1) as l2p:
            v = nc.vector
            g = nc.gpsimd

            cst = stp.tile([128, 8 * NC], F32, name="cst")
            nc.sync.dma_start(cst[:, :], cst_in[:, :])
            c1 = cst[:, 0 * NC:1 * NC]    # 1/s + margin
            c2 = cst[:, 1 * NC:2 * NC]    # ((1/K)^s)/s - margin
            sC = cst[:, 2 * NC:3 * NC]    # s
            pm1C = cst[:, 3 * NC:4 * NC]  # p-1
            pC = cst[:, 4 * NC:5 * NC]    # p
            c3 = cst[:, 5 * NC:6 * NC]    # 1 - 1/p
            rsC = cst[:, 6 * NC:7 * NC]   # 1/s
            mxC = cst[:, 7 * NC:8 * NC]   # host-computed per-row max

            mx = stp.tile([128, NC], F32, name="mx")
            tt = stp.tile([128, NC], F32, name="tt")   # current t
            lo = stp.tile([128, NC], F32, name="lo")
            hi = stp.tile([128, NC], F32, name="hi")
            Sv = stp.tile([128, NC], F32, name="Sv")   # raw sum (Σvu or Σw)
            Dv = stp.tile([128, NC], F32, name="Dv")
            hv = stp.tile([128, NC], F32, name="hv")
            t1 = stp.tile([128, NC], F32, name="t1")
            t2 = stp.tile([128, NC], F32, name="t2")
            rS = stp.tile([128, NC], F32, name="rS")
            m1 = stp.tile([128, NC], U8, name="m1")

            x16 = []
            for st in range(N_ST):
                x16.append(xrp.tile([128, R * K], F16, name="x16",
                                    tag=f"x16_{st}", bufs=1))

            def sb3(ap, kk=K):
                return ap.rearrange("p (j k) -> p j k", k=kk)

            def x_piece(handle, st, j):
                r0 = st * ST_ROWS + j * 128
                return handle[r0:r0 + 128, :]

            def fold_tree(src16, dst_cols, op, pool_l1=False):
                """src16: [128, R*K] fp16 -> per-subrow reduce into dst_cols.
                Two fp16 fold levels (L1 optionally on Pool) + DVE reduce."""
                l1t = l1p.tile([128, R * K // 2], F16, name="l1t")
                a = sb3(src16[:, :])
                l1a = sb3(l1t[:, :], K // 2)
                eng = g if pool_l1 else v
                eng.tensor_tensor(l1a[:, :, :], a[:, :, 0:K // 2],
                                  a[:, :, K // 2:K], op=op)
                l2t = l2p.tile([128, R * K // 4], F16, name="l2t")
                l2a = sb3(l2t[:, :], K // 4)
                v.tensor_tensor(l2a[:, :, :], l1a[:, :, 0:K // 4],
                                l1a[:, :, K // 4:K // 2], op=op)
                v.tensor_reduce(dst_cols, l2a[:, :, :],
                                axis=mybir.AxisListType.X, op=op)

            def do_clamp(st):
                cc = st * R
                u16 = up.tile([128, R * K], F16, name="u16")
                for j in range(R):
                    v.tensor_scalar(u16[:, j * K:(j + 1) * K],
                                    x16[st][:, j * K:(j + 1) * K],
                                    tt[:, cc + j:cc + j + 1], 0.0,
                                    op0=AL.subtract, op1=AL.max)
                return u16

            def eval_v(st, u16):
                cc = st * R
                c4 = slice(cc, cc + R)
                L = lp.tile([128, R * K], F32, name="L")
                nc.scalar.activation(L[:, :], u16[:, :], AF.Ln,
                                     scale=sC[:, cc:cc + 1])
                v16 = vp_.tile([128, R * K], F16, name="v16")
                for j in range(R):
                    nc.scalar.activation(v16[:, j * K:(j + 1) * K],
                                         L[:, j * K:(j + 1) * K], AF.Exp,
                                         scale=pm1C[:, cc:cc + 1],
                                         accum_out=Dv[:, cc + j:cc + j + 1])
                vu = vup.tile([128, R * K], F16, name="vu")
                for j in range(R):
                    eng = g if j < 2 else v
                    eng.tensor_tensor(vu[:, j * K:(j + 1) * K],
                                      v16[:, j * K:(j + 1) * K],
                                      u16[:, j * K:(j + 1) * K], op=AL.mult)
                fold_tree(vu, Sv[:, c4], AL.add)

            def eval_final(st, u16):
                cc = st * R
                c4 = slice(cc, cc + R)
                L = lp.tile([128, R * K], F32, name="L")
                nc.scalar.activation(L[:, :], u16[:, :], AF.Ln,
                                     scale=sC[:, cc:cc + 1])
                w16 = vp_.tile([128, R * K], F16, name="v16")
                nc.scalar.activation(w16[:, :], L[:, :], AF.Exp,
                                     scale=pC[:, cc:cc + 1])
                fold_tree(w16, Sv[:, c4], AL.add)
                v.reciprocal(rS[:, c4], Sv[:, c4])
                for j in range(R):
                    yq = sgp.tile([128, K], F32, name="stg")
                    v.tensor_scalar(yq[:, :], w16[:, j * K:(j + 1) * K],
                                    rS[:, cc + j:cc + j + 1], None,
                                    op0=AL.mult)
                    nc.sync.dma_start(x_piece(y_out, st, j), yq[:, :])

            def state_update(csl, first):
                """csl: [a,b) column slice of active update slots."""
                nc.scalar.activation(hv[:, csl], Sv[:, csl], AF.Ln)
                v.tensor_scalar(m1[:, csl], hv[:, csl], 0.0, None, op0=AL.is_ge)
                v.copy_predicated(lo[:, csl], m1[:, csl], tt[:, csl])
                if first:
                    # hi = min(hi, t + h)
                    v.tensor_tensor(t1[:, csl], tt[:, csl], hv[:, csl],
                                    op=AL.add)
                    v.tensor_tensor(hi[:, csl], hi[:, csl], t1[:, csl],
                                    op=AL.min)
                v.tensor_scalar(m1[:, csl], hv[:, csl], 0.0, None, op0=AL.is_lt)
                v.copy_predicated(hi[:, csl], m1[:, csl], tt[:, csl])
                v.tensor_tensor(t1[:, csl], c3[:, csl], hv[:, csl], op=AL.mult)
                nc.scalar.activation(t1[:, csl], t1[:, csl], AF.Exp)
                v.tensor_tensor(t1[:, csl], Sv[:, csl], t1[:, csl],
                                op=AL.subtract)   # S - S^(1-1/p)
                v.tensor_scalar(t2[:, csl], Dv[:, csl], 1e-30, None, op0=AL.max)
                v.reciprocal(t2[:, csl], t2[:, csl])
                v.tensor_tensor(t1[:, csl], t1[:, csl], t2[:, csl], op=AL.mult)
                v.tensor_tensor(t1[:, csl], t1[:, csl], rsC[:, csl], op=AL.mult)
                v.tensor_tensor(tt[:, csl], tt[:, csl], t1[:, csl], op=AL.add)
                v.tensor_tensor(tt[:, csl], tt[:, csl], lo[:, csl], op=AL.max)
                v.tensor_tensor(t1[:, csl], hi[:, csl], lo[:, csl], op=AL.max)
                v.tensor_tensor(tt[:, csl], tt[:, csl], t1[:, csl], op=AL.min)

            # ---- init: brackets from host-side maxes, then pipelined
            # load/convert/eval0 with clamps emitted one supertile ahead
            # (keeps ACT streaming; DVE's in-order queue never blocks it)
            v.tensor_tensor(lo[:, :], mxC[:, :], c1[:, :], op=AL.subtract)
            v.tensor_tensor(hi[:, :], mxC[:, :], c2[:, :], op=AL.subtract)
            v.tensor_copy(tt[:, :], lo[:, :])

            def load_conv(st):
                for j in range(R):
                    stg = sgp.tile([128, K], F32, name="stg")
                    nc.sync.dma_start(stg[:, :], x_piece(x_in, st, j))
                    v.tensor_copy(x16[st][:, j * K:(j + 1) * K], stg[:, :])

            load_conv(0)
            u_pend = do_clamp(0)
            for st in range(N_ST):
                if st + 1 < N_ST:
                    load_conv(st + 1)
                u_cur = u_pend
                if st + 1 < N_ST:
                    u_pend = do_clamp(st + 1)
                eval_v(st, u_cur)
            ha = slice(0, NC // 2)
            hb = slice(NC // 2, NC)
            v.tensor_tensor(Sv[:, ha], Sv[:, ha], sC[:, ha], op=AL.mult)
            state_update(ha, first=True)
            u_pend = do_clamp(0) if emax_r > 1 else None
            v.tensor_tensor(Sv[:, hb], Sv[:, hb], sC[:, hb], op=AL.mult)
            state_update(hb, first=True)

            # ---- rounds (E descending across slots: active update slots
            # are a global prefix -> one batched state update per round)
            for r in range(1, emax_r):
                active = [st for st in range(N_ST) if r <= e_slots[st] - 1]
                upd = [st for st in active if r < e_slots[st] - 1]
                for i, st in enumerate(active):
                    u_cur = u_pend if u_pend is not None else do_clamp(st)
                    u_pend = (do_clamp(active[i + 1])
                              if i + 1 < len(active) else None)
                    if r < e_slots[st] - 1:
                        eval_v(st, u_cur)
                    else:
                        eval_final(st, u_cur)
                if upd:
                    assert upd == list(range(len(upd))), (r, upd)
                    n1 = (len(upd) + 1) // 2
                    ca = slice(0, n1 * R)
                    cb = slice(n1 * R, len(upd) * R)
                    v.tensor_tensor(Sv[:, ca], Sv[:, ca], sC[:, ca],
                                    op=AL.mult)
                    state_update(ca, first=False)
                    if r + 1 < emax_r:
                        u_pend = do_clamp(0)
                    if cb.start < cb.stop:
                        v.tensor_tensor(Sv[:, cb], Sv[:, cb], sC[:, cb],
                                        op=AL.mult)
                        state_update(cb, first=False)

    orig_tables = bacc.get_activation_tables

    def _lnexp_only(arch):
        return {k: (val if k == "natural_log_exp_and_others" else set())
                for k, val in orig_tables(arch).items()}

    bacc.get_activation_tables = _lnexp_only
    try:
        nc.finalize()
    finally:
        bacc.get_activation_tables = orig_tables
    return nc


# -------------------------------------------------------------------- driver

def kernel(att_scores: np.ndarray, alpha: np.ndarray) -> np.ndarray:
    global LAST_RESULT
    X = np.ascontiguousarray(np.asarray(att_scores, dtype=np.float32))
    X4 = X.reshape(B, H, Q, K)
    al = np.minimum(np.asarray(alpha, dtype=np.float64).reshape(H), 2.0 - 1e-4)

    E = _choose_E(X4, al)

    # block list per E-class: class e -> [(b, h)] ; deal round-robin to cores
    order = np.argsort(-E, kind='stable')       # heads by E desc
    core_blocks = [[] for _ in range(NCORES)]
    i = 0
    for h in order:
        for b in range(B):
            core_blocks[i % NCORES].append((b, h))
            i += 1
    # within each core blocks are appended in E-desc order already
    e_slots = []
    for (b, h) in core_blocks[0]:
        e_slots += [int(E[h])] * 2
    e_slots = tuple(e_slots)
    for cb in core_blocks:
        es = []
        for (b, h) in cb:
            es += [int(E[h])] * 2
        assert tuple(es) == e_slots, "unbalanced core E profile"

    if e_slots not in _NC_CACHE:
        _NC_CACHE[e_slots] = _build(e_slots)
    nc = _NC_CACHE[e_slots]

    in_maps = []
    for c in range(NCORES):
        xc = np.concatenate([X4[b, h] for (b, h) in core_blocks[c]], axis=0)
        cvec = np.zeros((7, NC), np.float64)
        xc2 = np.ascontiguousarray(xc.reshape(BLOCKS * Q, K))
        mx_rows = xc2.max(-1)                      # [8192]
        mx_arr = mx_rows.reshape(N_ST, R, 128).transpose(2, 0, 1).reshape(128, NC)
        for sl in range(BLOCKS):
            h = core_blocks[c][sl][1]
            s = al[h] - 1.0
            cols = slice(sl * 2 * R, (sl + 1) * 2 * R)
            cvec[0, cols] = 1.0 / s + MARGIN
            cvec[1, cols] = ((1.0 / K) ** s) / s - MARGIN
            cvec[2, cols] = s
            cvec[3, cols] = 1.0 / s - 1.0
            cvec[4, cols] = 1.0 / s
            cvec[5, cols] = 1.0 - s
            cvec[6, cols] = 1.0 / s
        cst = np.concatenate(
            [np.tile(cvec.reshape(1, 7 * NC).astype(np.float32), (128, 1)),
             mx_arr.astype(np.float32)], axis=1)
        in_maps.append({"x": xc2, "cst": cst})

    res = run_bass_kernel_spmd(nc, in_maps, core_ids=list(range(NCORES)))
    LAST_RESULT = res

    out = np.empty((B, H, Q, K), np.float32)
    for c in range(NCORES):
        yc = np.asarray(res.results[c]["y"]).reshape(BLOCKS, Q, K)
        for sl, (b, h) in enumerate(core_blocks[c]):
            out[b, h] = yc[sl]
    return out


def _get_nc():
    return next(iter(_NC_CACHE.values())) if _NC_CACHE else None
